# revision 27
# baseline (speedup 1.0000x reference)
"""Trainium2 Bass kernel for ContMultiHeadedAttention.

Full (unsharded) inputs in, full output out. Sharding: tensor-parallel over
the 8 heads — core c computes head c for both batches and the corresponding
slice of the output projection; the host sums the 8 partial outputs
(row-parallel linear unshard).

Math per head h (performed transposed so no on-device transposes of the
attention matrix are needed):
  qpT = (Wq_h/8).T @ q.T + bq_h/8        [64, S]   (fp16)
  kpT = Wk_h.T @ k.T + bk_h              [64, S]   (fp16)
  vp  = v @ Wv_h + bv_h, augmented with a ones column -> [S, 65]  (fp16)
  sT[k,q] = kpT.T @ qpT  (+ biasT, mask -> -1e4)   (fp32 psum)
  p = exp(sT)  (fp16; no max-subtraction needed: |s| is bounded ~10)
  ctxT[0:64, q] = vp.T @ p (unnormalized), ctxT[64, q] = rowsum
  y = ctxT[0:64].T @ Wo_h + rowsum*bo  then divided by rowsum  [S, 512]
"""

import os
import sys
import types
import numpy as np

B = 2
S = 2048
F = 512          # model dim
H = 8            # heads
D = 64           # head dim
DV = 65          # head dim + ones column
KC = 16          # key chunks of 128 partitions
QC = 4           # query chunks of 512
FC = 4           # feature chunks of 128
N_CORES = 8
NEG = -10000.0   # effective -inf for masked scores (exp underflows to 0)


def _install_ntff_hook():
    """Recreate antenv.axon_hooks if the image lacks it so trace=True works."""
    try:
        import antenv
        if "antenv.axon_hooks" in sys.modules:
            return
        mod = types.ModuleType("antenv.axon_hooks")
        _h = [None]
        mod.set_axon_ntff_profile_hook = lambda h: _h.__setitem__(0, h)
        mod.get_axon_ntff_profile_hook = lambda: _h[0]
        sys.modules["antenv.axon_hooks"] = mod
        antenv.axon_hooks = mod
        from trn_agent_boot.trn_boot import _ntff_profile_via_ctypes
        mod.set_axon_ntff_profile_hook(
            _ntff_profile_via_ctypes("/opt/axon/libaxon_pjrt.so")
        )
    except Exception:
        pass


_PROGRAM = None


def _build_program():
    global _PROGRAM
    if _PROGRAM is not None:
        return _PROGRAM

    import concourse.bacc as bacc
    import concourse.tile as tile
    from concourse import mybir

    f32 = mybir.dt.float32
    f16 = mybir.dt.float16
    u8 = mybir.dt.uint8
    AF = mybir.ActivationFunctionType

    nc = bacc.Bacc("TRN2", target_bir_lowering=False, debug=False,
                   enable_asserts=True, num_devices=N_CORES)

    xq = nc.dram_tensor("xq", [B, F, S], f16, kind="ExternalInput").ap()
    xk = nc.dram_tensor("xk", [B, F, S], f16, kind="ExternalInput").ap()
    xv = nc.dram_tensor("xv", [B, F, S], f16, kind="ExternalInput").ap()
    # biasT[k,q] = bias[q,k] where mask[q,k] else -1e4 (mask folded host-side
    # into the two constant inputs; exp underflows masked scores to 0)
    biasT = nc.dram_tensor("biasT", [S, S], f16, kind="ExternalInput").ap()
    wq_d = nc.dram_tensor("wq", [F, D], f16, kind="ExternalInput").ap()
    wk_d = nc.dram_tensor("wk", [F, D], f16, kind="ExternalInput").ap()
    wv_d = nc.dram_tensor("wv", [F, DV], f16, kind="ExternalInput").ap()
    bq_d = nc.dram_tensor("bq", [D, 1], f32, kind="ExternalInput").ap()
    bk_d = nc.dram_tensor("bk", [D, 1], f32, kind="ExternalInput").ap()
    bv_d = nc.dram_tensor("bv", [1, DV], f32, kind="ExternalInput").ap()
    # wo_aug: [65, F] fp16, row 64 = bo (multiplied by the ones row of cn)
    wo_d = nc.dram_tensor("wo", [DV, F], f16, kind="ExternalInput").ap()
    y_d = nc.dram_tensor("y", [B, S, F], f32, kind="ExternalOutput").ap()

    with tile.TileContext(nc) as tc:
        from contextlib import ExitStack
        with ExitStack() as ctx:
            consts = ctx.enter_context(tc.tile_pool(name="consts", bufs=1))
            persist = ctx.enter_context(tc.tile_pool(name="persist", bufs=1))
            xin = ctx.enter_context(tc.tile_pool(name="xin", bufs=3))
            bmp = ctx.enter_context(tc.tile_pool(name="bmp", bufs=2))
            mkp = ctx.enter_context(tc.tile_pool(name="mkp", bufs=2))
            bms_p = ctx.enter_context(tc.tile_pool(name="bms_p", bufs=4))
            tp = ctx.enter_context(tc.tile_pool(name="tp", bufs=4))
            pp = ctx.enter_context(tc.tile_pool(name="pp", bufs=6))
            cnp = ctx.enter_context(tc.tile_pool(name="cnp", bufs=2))
            rsp = ctx.enter_context(tc.tile_pool(name="rsp", bufs=2))
            rbp = ctx.enter_context(tc.tile_pool(name="rbp", bufs=2))
            psA = ctx.enter_context(tc.tile_pool(name="psA", bufs=2, space="PSUM"))
            psC = ctx.enter_context(tc.tile_pool(name="psC", bufs=1, space="PSUM"))
            psY = ctx.enter_context(tc.tile_pool(name="psY", bufs=1, space="PSUM"))
            psV = ctx.enter_context(tc.tile_pool(name="psV", bufs=1, space="PSUM"))

            # ---- weights / constants in SBUF ----
            wq_sb = consts.tile([128, FC, D], f16, tag="wq")
            nc.sync.dma_start(out=wq_sb[:], in_=wq_d.rearrange("(c p) d -> p c d", p=128))
            wk_sb = consts.tile([128, FC, D], f16, tag="wk")
            nc.sync.dma_start(out=wk_sb[:], in_=wk_d.rearrange("(c p) d -> p c d", p=128))
            wv_sb = consts.tile([128, FC, DV], f16, tag="wv")
            nc.sync.dma_start(out=wv_sb[:], in_=wv_d.rearrange("(c p) d -> p c d", p=128))
            bq_sb = consts.tile([D, 1], f32, tag="bq")
            nc.sync.dma_start(out=bq_sb[:], in_=bq_d[:])
            bk_sb = consts.tile([D, 1], f32, tag="bk")
            nc.sync.dma_start(out=bk_sb[:], in_=bk_d[:])
            wo_sb = consts.tile([DV, F], f16, tag="wo")
            nc.sync.dma_start(out=wo_sb[:], in_=wo_d[:])
            # ones row + bv on the same partition base (K=1 matmuls need
            # lhsT and rhs on the same physical partitions)
            vbias_row = consts.tile([1, 128 + DV], f32, tag="vbias_row")
            nc.gpsimd.memset(vbias_row[:], 1.0)
            nc.sync.dma_start(out=vbias_row[:, 128:128 + DV], in_=bv_d[:])
            ones_row = vbias_row[:, 0:128]
            bv_sb = vbias_row[:, 128:128 + DV]

            qp = {}
            kp = {}
            vp = {}
            for b in range(B):
                qp[b] = persist.tile([D, S], f16, tag=f"qp{b}", name=f"qp{b}")
                kp[b] = persist.tile([D, S], f16, tag=f"kp{b}", name=f"kp{b}")
                vp[b] = persist.tile([128, KC * DV], f16, tag=f"vp{b}", name=f"vp{b}")

            # ---- phase 1: projections ----
            for b in range(B):
                for x_d, w_sb, b_sb, dst in (
                    (xq, wq_sb, bq_sb, qp[b]),
                    (xk, wk_sb, bk_sb, kp[b]),
                ):
                    xt = xin.tile([128, FC, S], f16, tag="xin")
                    nc.sync.dma_start(
                        out=xt[:], in_=x_d[b].rearrange("(c p) s -> p c s", p=128)
                    )
                    for sc in range(QC):
                        ps = psA.tile([D, 512], f32, tag="s")
                        for fc in range(FC):
                            nc.tensor.matmul(
                                ps[:],
                                lhsT=w_sb[:, fc, :],
                                rhs=xt[:, fc, sc * 512:(sc + 1) * 512],
                                start=(fc == 0),
                                stop=(fc == FC - 1),
                            )
                        nc.scalar.activation(
                            dst[:, sc * 512:(sc + 1) * 512], ps[:],
                            AF.Identity, bias=b_sb[:],
                        )
                # v projection -> vp [s(128-chunks), 65] with ones column
                xt = xin.tile([128, FC, S], f16, tag="xin")
                nc.sync.dma_start(
                    out=xt[:], in_=xv[b].rearrange("(c p) s -> p c s", p=128)
                )
                for sc in range(KC):
                    ps = psV.tile([128, DV], f32, tag="v", name="psv")
                    for fc in range(FC):
                        nc.tensor.matmul(
                            ps[:],
                            lhsT=xt[:, fc, sc * 128:(sc + 1) * 128],
                            rhs=wv_sb[:, fc, :],
                            start=(fc == 0),
                            stop=False,
                        )
                    nc.tensor.matmul(
                        ps[:], lhsT=ones_row[:], rhs=bv_sb[:],
                        start=False, stop=True,
                    )
                    nc.scalar.activation(
                        vp[b][:, sc * DV:(sc + 1) * DV], ps[:], AF.Copy
                    )

            # ---- phase 2+3: attention + output projection ----
            for qc in range(QC):
                q0 = qc * 512
                bmt = bmp.tile([128, KC * 512], f16, tag="bm")
                nc.sync.dma_start(
                    out=bmt[:],
                    in_=biasT.rearrange("(kc p) q -> p kc q", p=128)[:, :, q0:q0 + 512],
                )
                ctxps = {}
                for b in range(B):
                    ctxps[b] = psC.tile([DV, 512], f32, tag=f"ctx{b}", name=f"ctx{b}")
                for kc in range(KC):
                    # both batches' scores packed in one 2-bank psum tile
                    st2 = psA.tile([128, B * 512], f32, tag="s")
                    for b in range(B):
                        nc.tensor.matmul(
                            st2[:, b * 512:(b + 1) * 512],
                            lhsT=kp[b][:, kc * 128:(kc + 1) * 128],
                            rhs=qp[b][:, q0:q0 + 512],
                            start=True, stop=True,
                        )
                    tt2 = tp.tile([128, B * 512], f32, tag="t")
                    pt2 = pp.tile([128, B * 512], f16, tag="p")
                    for b in range(B):
                        nc.vector.tensor_add(
                            tt2[:, b * 512:(b + 1) * 512],
                            st2[:, b * 512:(b + 1) * 512],
                            bmt[:, kc * 512:(kc + 1) * 512],
                        )
                        nc.scalar.activation(
                            pt2[:, b * 512:(b + 1) * 512],
                            tt2[:, b * 512:(b + 1) * 512], AF.Exp,
                        )
                    for b in range(B):
                        nc.tensor.matmul(
                            ctxps[b][:],
                            lhsT=vp[b][:, kc * DV:(kc + 1) * DV],
                            rhs=pt2[:, b * 512:(b + 1) * 512],
                            start=(kc == 0),
                            stop=(kc == KC - 1),
                        )
                for b in range(B):
                    # normalize ctx by the rowsum (row 64), then fp16 Wo matmul
                    rr = rsp.tile([1, 512], f32, tag="rr")
                    nc.vector.reciprocal(rr[:], ctxps[b][D:DV, :])
                    rbc = rbp.tile([D, 512], f32, tag="rbc")
                    nc.gpsimd.partition_broadcast(rbc[:], rr[:], channels=D)
                    cn = cnp.tile([DV, 512], f16, tag="cn")
                    nc.vector.tensor_mul(cn[0:D, :], ctxps[b][0:D, :], rbc[:])
                    nc.gpsimd.memset(cn[D:DV, :], 1.0)
                    for s4 in range(4):
                        sl = slice(s4 * 128, (s4 + 1) * 128)
                        yps = psY.tile([128, 512], f32, tag="y")
                        nc.tensor.matmul(
                            yps[:], lhsT=cn[:, sl], rhs=wo_sb[:],
                            start=True, stop=True,
                        )
                        yt = tp.tile([128, 512], f32, tag="t")
                        if b == 0:
                            nc.scalar.activation(yt[:], yps[:], AF.Copy)
                        else:
                            nc.vector.tensor_copy(yt[:], yps[:])
                        nc.sync.dma_start(
                            out=y_d[b, q0 + s4 * 128:q0 + (s4 + 1) * 128, :],
                            in_=yt[:],
                        )

    nc.compile()
    _PROGRAM = nc
    return nc


def _prep_inputs(k, v, q, mask, spatial_bias, Wq, bq, Wk, bk, Wv, bv, Wo, bo):
    """Build the 8 per-core input maps (host-side sharding / layout only)."""
    f16 = np.float16
    qT = np.ascontiguousarray(np.transpose(q, (0, 2, 1)).astype(f16))
    kT = np.ascontiguousarray(np.transpose(k, (0, 2, 1)).astype(f16))
    vT = np.ascontiguousarray(np.transpose(v, (0, 2, 1)).astype(f16))
    maskT = mask.T
    negf = np.float16(NEG)

    in_maps = []
    for h in range(N_CORES):
        sl = slice(h * D, (h + 1) * D)
        wv_aug = np.concatenate(
            [Wv[:, sl], np.zeros((F, 1), np.float32)], axis=1
        ).astype(f16)
        bv_aug = np.concatenate([bv[sl], [1.0]]).astype(np.float32).reshape(1, DV)
        bo_h = bo if h == 0 else np.zeros_like(bo)
        wo_aug = np.concatenate(
            [Wo[sl, :], bo_h.reshape(1, F)], axis=0
        ).astype(f16)
        in_maps.append({
            "xq": qT, "xk": kT, "xv": vT,
            "biasT": np.where(maskT, spatial_bias[0, h].T.astype(f16), negf),
            "wq": (Wq[:, sl] / 8.0).astype(f16),
            "wk": Wk[:, sl].astype(f16),
            "wv": wv_aug,
            "bq": (bq[sl] / 8.0).astype(np.float32).reshape(D, 1),
            "bk": bk[sl].astype(np.float32).reshape(D, 1),
            "bv": bv_aug,
            "wo": wo_aug,
        })
    return in_maps


LAST_EXEC_NS = None
LAST_TRACE = None


def kernel(**inputs) -> np.ndarray:
    global LAST_EXEC_NS, LAST_TRACE
    trace = bool(int(os.environ.get("KERNEL_TRACE", "0")))
    if trace:
        _install_ntff_hook()
    from concourse.bass_utils import run_bass_kernel_spmd

    nc = _build_program()
    in_maps = _prep_inputs(**{k: np.asarray(v) for k, v in inputs.items()})
    res = run_bass_kernel_spmd(
        nc, in_maps, core_ids=list(range(N_CORES)), trace=trace
    )
    LAST_EXEC_NS = res.exec_time_ns
    LAST_TRACE = res.instructions_and_trace[1] if res.instructions_and_trace else None
    out = res.results[0]["y"].astype(np.float32)
    for c in range(1, N_CORES):
        out += res.results[c]["y"]
    return out


# revision 28
# speedup vs baseline: 1.1615x; 1.1615x over previous
"""Trainium2 Bass kernel for ContMultiHeadedAttention.

Full (unsharded) inputs in, full output out. Sharding: tensor-parallel over
the 8 heads — core c computes head c for both batches and the corresponding
slice of the output projection; the host sums the 8 partial outputs
(row-parallel linear unshard).

Math per head h (performed transposed so no on-device transposes of the
attention matrix are needed):
  qpT = (Wq_h/8).T @ q.T + bq_h/8        [64, S]   (fp16)
  kpT = Wk_h.T @ k.T + bk_h              [64, S]   (fp16)
  vp  = v @ Wv_h + bv_h, augmented with a ones column -> [S, 65]  (fp16)
  sT[k,q] = kpT.T @ qpT  (+ biasT, mask -> -1e4)   (fp32 psum)
  p = exp(sT)  (fp16; no max-subtraction needed: |s| is bounded ~10)
  ctxT[0:64, q] = vp.T @ p (unnormalized), ctxT[64, q] = rowsum
  y = ctxT[0:64].T @ Wo_h + rowsum*bo  then divided by rowsum  [S, 512]
"""

import os
import sys
import types
import numpy as np

B = 2
S = 2048
F = 512          # model dim
H = 8            # heads
D = 64           # head dim
DV = 65          # head dim + ones column
KC = 16          # key chunks of 128 partitions
QC = 4           # query chunks of 512
FC = 4           # feature chunks of 128
N_CORES = 8
NEG = -10000.0   # effective -inf for masked scores (exp underflows to 0)


def _install_ntff_hook():
    """Recreate antenv.axon_hooks if the image lacks it so trace=True works."""
    try:
        import antenv
        if "antenv.axon_hooks" in sys.modules:
            return
        mod = types.ModuleType("antenv.axon_hooks")
        _h = [None]
        mod.set_axon_ntff_profile_hook = lambda h: _h.__setitem__(0, h)
        mod.get_axon_ntff_profile_hook = lambda: _h[0]
        sys.modules["antenv.axon_hooks"] = mod
        antenv.axon_hooks = mod
        from trn_agent_boot.trn_boot import _ntff_profile_via_ctypes
        mod.set_axon_ntff_profile_hook(
            _ntff_profile_via_ctypes("/opt/axon/libaxon_pjrt.so")
        )
    except Exception:
        pass


_PROGRAM = None


def _build_program():
    global _PROGRAM
    if _PROGRAM is not None:
        return _PROGRAM

    import concourse.bacc as bacc
    import concourse.tile as tile
    from concourse import mybir

    f32 = mybir.dt.float32
    f16 = mybir.dt.float16
    u8 = mybir.dt.uint8
    AF = mybir.ActivationFunctionType

    nc = bacc.Bacc("TRN2", target_bir_lowering=False, debug=False,
                   enable_asserts=True, num_devices=N_CORES)

    xq = nc.dram_tensor("xq", [B, F, S], f16, kind="ExternalInput").ap()
    xk = nc.dram_tensor("xk", [B, F, S], f16, kind="ExternalInput").ap()
    xv = nc.dram_tensor("xv", [B, F, S], f16, kind="ExternalInput").ap()
    # biasT[k,q] = bias[q,k] where mask[q,k] else -1e4 (mask folded host-side
    # into the two constant inputs; exp underflows masked scores to 0)
    biasT = nc.dram_tensor("biasT", [S, S], f16, kind="ExternalInput").ap()
    wq_d = nc.dram_tensor("wq", [F, D], f16, kind="ExternalInput").ap()
    wk_d = nc.dram_tensor("wk", [F, D], f16, kind="ExternalInput").ap()
    wv_d = nc.dram_tensor("wv", [F, DV], f16, kind="ExternalInput").ap()
    bq_d = nc.dram_tensor("bq", [D, 1], f32, kind="ExternalInput").ap()
    bk_d = nc.dram_tensor("bk", [D, 1], f32, kind="ExternalInput").ap()
    bv_d = nc.dram_tensor("bv", [1, DV], f32, kind="ExternalInput").ap()
    # wo_aug: [65, F] fp16, row 64 = bo (multiplied by the ones row of cn)
    wo_d = nc.dram_tensor("wo", [DV, F], f16, kind="ExternalInput").ap()
    y_d = nc.dram_tensor("y", [B, S, F], f32, kind="ExternalOutput").ap()

    with tile.TileContext(nc) as tc:
        from contextlib import ExitStack
        with ExitStack() as ctx:
            consts = ctx.enter_context(tc.tile_pool(name="consts", bufs=1))
            persist = ctx.enter_context(tc.tile_pool(name="persist", bufs=1))
            xin = ctx.enter_context(tc.tile_pool(name="xin", bufs=3))
            bmp = ctx.enter_context(tc.tile_pool(name="bmp", bufs=2))
            mkp = ctx.enter_context(tc.tile_pool(name="mkp", bufs=2))
            bms_p = ctx.enter_context(tc.tile_pool(name="bms_p", bufs=4))
            tp = ctx.enter_context(tc.tile_pool(name="tp", bufs=4))
            pp = ctx.enter_context(tc.tile_pool(name="pp", bufs=6))
            cnp = ctx.enter_context(tc.tile_pool(name="cnp", bufs=2))
            rsp = ctx.enter_context(tc.tile_pool(name="rsp", bufs=2))
            rbp = ctx.enter_context(tc.tile_pool(name="rbp", bufs=2))
            psA = ctx.enter_context(tc.tile_pool(name="psA", bufs=2, space="PSUM"))
            psC = ctx.enter_context(tc.tile_pool(name="psC", bufs=1, space="PSUM"))
            psY = ctx.enter_context(tc.tile_pool(name="psY", bufs=2, space="PSUM"))

            # ---- weights / constants in SBUF ----
            wq_sb = consts.tile([128, FC, D], f16, tag="wq")
            nc.sync.dma_start(out=wq_sb[:], in_=wq_d.rearrange("(c p) d -> p c d", p=128))
            wk_sb = consts.tile([128, FC, D], f16, tag="wk")
            nc.sync.dma_start(out=wk_sb[:], in_=wk_d.rearrange("(c p) d -> p c d", p=128))
            wv_sb = consts.tile([128, FC, DV], f16, tag="wv")
            nc.sync.dma_start(out=wv_sb[:], in_=wv_d.rearrange("(c p) d -> p c d", p=128))
            bq_sb = consts.tile([D, 1], f32, tag="bq")
            nc.sync.dma_start(out=bq_sb[:], in_=bq_d[:])
            bk_sb = consts.tile([D, 1], f32, tag="bk")
            nc.sync.dma_start(out=bk_sb[:], in_=bk_d[:])
            wo_sb = consts.tile([DV, F], f16, tag="wo")
            nc.sync.dma_start(out=wo_sb[:], in_=wo_d[:])
            # ones row + bv on the same partition base (K=1 matmuls need
            # lhsT and rhs on the same physical partitions)
            vbias_row = consts.tile([1, 128 + DV], f32, tag="vbias_row")
            nc.gpsimd.memset(vbias_row[:], 1.0)
            nc.sync.dma_start(out=vbias_row[:, 128:128 + DV], in_=bv_d[:])
            ones_row = vbias_row[:, 0:128]
            bv_sb = vbias_row[:, 128:128 + DV]

            qp = {}
            kp = {}
            vp = {}
            for b in range(B):
                qp[b] = persist.tile([D, S], f16, tag=f"qp{b}", name=f"qp{b}")
                kp[b] = persist.tile([D, S], f16, tag=f"kp{b}", name=f"kp{b}")
                vp[b] = persist.tile([128, KC * DV], f16, tag=f"vp{b}", name=f"vp{b}")

            # ---- phase 1: projections ----
            for b in range(B):
                for x_d, w_sb, b_sb, dst in (
                    (xq, wq_sb, bq_sb, qp[b]),
                    (xk, wk_sb, bk_sb, kp[b]),
                ):
                    xt = xin.tile([128, FC, S], f16, tag="xin")
                    nc.sync.dma_start(
                        out=xt[:], in_=x_d[b].rearrange("(c p) s -> p c s", p=128)
                    )
                    for sc in range(QC):
                        ps = psA.tile([D, 512], f32, tag="s")
                        for fc in range(FC):
                            nc.tensor.matmul(
                                ps[:],
                                lhsT=w_sb[:, fc, :],
                                rhs=xt[:, fc, sc * 512:(sc + 1) * 512],
                                start=(fc == 0),
                                stop=(fc == FC - 1),
                            )
                        nc.scalar.activation(
                            dst[:, sc * 512:(sc + 1) * 512], ps[:],
                            AF.Identity, bias=b_sb[:],
                        )
                # v projection -> vp [s(128-chunks), 65] with ones column
                xt = xin.tile([128, FC, S], f16, tag="xin")
                nc.sync.dma_start(
                    out=xt[:], in_=xv[b].rearrange("(c p) s -> p c s", p=128)
                )
                for sc in range(KC):
                    ps = psY.tile([128, DV], f32, tag="y", name="psv")
                    for fc in range(FC):
                        nc.tensor.matmul(
                            ps[:],
                            lhsT=xt[:, fc, sc * 128:(sc + 1) * 128],
                            rhs=wv_sb[:, fc, :],
                            start=(fc == 0),
                            stop=False,
                        )
                    nc.tensor.matmul(
                        ps[:], lhsT=ones_row[:], rhs=bv_sb[:],
                        start=False, stop=True,
                    )
                    nc.scalar.activation(
                        vp[b][:, sc * DV:(sc + 1) * DV], ps[:], AF.Copy
                    )

            # ---- phase 2+3: attention + output projection ----
            for qc in range(QC):
                q0 = qc * 512
                bmt = bmp.tile([128, KC * 512], f16, tag="bm")
                nc.sync.dma_start(
                    out=bmt[:],
                    in_=biasT.rearrange("(kc p) q -> p kc q", p=128)[:, :, q0:q0 + 512],
                )
                ctxps = {}
                for b in range(B):
                    ctxps[b] = psC.tile([DV, 512], f32, tag=f"ctx{b}", name=f"ctx{b}")
                for kc in range(KC):
                    # both batches' scores packed in one 2-bank psum tile
                    st2 = psA.tile([128, B * 512], f32, tag="s")
                    for b in range(B):
                        nc.tensor.matmul(
                            st2[:, b * 512:(b + 1) * 512],
                            lhsT=kp[b][:, kc * 128:(kc + 1) * 128],
                            rhs=qp[b][:, q0:q0 + 512],
                            start=True, stop=True,
                        )
                    bms_rep = (
                        bmt[:, kc * 512:(kc + 1) * 512]
                        .rearrange("p (o q) -> p o q", o=1)
                        .broadcast_to((128, B, 512))
                    )
                    tt2 = tp.tile([128, B * 512], f32, tag="t")
                    nc.vector.tensor_add(
                        tt2[:].rearrange("p (o q) -> p o q", o=B),
                        st2[:].rearrange("p (o q) -> p o q", o=B),
                        bms_rep,
                    )
                    pt2 = pp.tile([128, B * 512], f16, tag="p")
                    nc.scalar.activation(pt2[:], tt2[:], AF.Exp)
                    for b in range(B):
                        nc.tensor.matmul(
                            ctxps[b][:],
                            lhsT=vp[b][:, kc * DV:(kc + 1) * DV],
                            rhs=pt2[:, b * 512:(b + 1) * 512],
                            start=(kc == 0),
                            stop=(kc == KC - 1),
                        )
                for b in range(B):
                    # normalize ctx by the rowsum (row 64), then fp16 Wo matmul
                    rr = rsp.tile([1, 512], f32, tag="rr")
                    nc.vector.reciprocal(rr[:], ctxps[b][D:DV, :])
                    rbc = rbp.tile([D, 512], f32, tag="rbc")
                    nc.gpsimd.partition_broadcast(rbc[:], rr[:], channels=D)
                    cn = cnp.tile([DV, 512], f16, tag="cn")
                    nc.vector.tensor_mul(cn[0:D, :], ctxps[b][0:D, :], rbc[:])
                    nc.gpsimd.memset(cn[D:DV, :], 1.0)
                    for s4 in range(4):
                        sl = slice(s4 * 128, (s4 + 1) * 128)
                        yps = psY.tile([128, 512], f32, tag="y")
                        nc.tensor.matmul(
                            yps[:], lhsT=cn[:, sl], rhs=wo_sb[:],
                            start=True, stop=True,
                        )
                        yt = tp.tile([128, 512], f32, tag="t")
                        if b == 0:
                            nc.scalar.activation(yt[:], yps[:], AF.Copy)
                        else:
                            nc.vector.tensor_copy(yt[:], yps[:])
                        nc.sync.dma_start(
                            out=y_d[b, q0 + s4 * 128:q0 + (s4 + 1) * 128, :],
                            in_=yt[:],
                        )

    nc.compile()
    _PROGRAM = nc
    return nc


def _prep_inputs(k, v, q, mask, spatial_bias, Wq, bq, Wk, bk, Wv, bv, Wo, bo):
    """Build the 8 per-core input maps (host-side sharding / layout only)."""
    f16 = np.float16
    qT = np.ascontiguousarray(np.transpose(q, (0, 2, 1)).astype(f16))
    kT = np.ascontiguousarray(np.transpose(k, (0, 2, 1)).astype(f16))
    vT = np.ascontiguousarray(np.transpose(v, (0, 2, 1)).astype(f16))
    maskT = mask.T
    negf = np.float16(NEG)

    in_maps = []
    for h in range(N_CORES):
        sl = slice(h * D, (h + 1) * D)
        wv_aug = np.concatenate(
            [Wv[:, sl], np.zeros((F, 1), np.float32)], axis=1
        ).astype(f16)
        bv_aug = np.concatenate([bv[sl], [1.0]]).astype(np.float32).reshape(1, DV)
        bo_h = bo if h == 0 else np.zeros_like(bo)
        wo_aug = np.concatenate(
            [Wo[sl, :], bo_h.reshape(1, F)], axis=0
        ).astype(f16)
        in_maps.append({
            "xq": qT, "xk": kT, "xv": vT,
            "biasT": np.where(maskT, spatial_bias[0, h].T.astype(f16), negf),
            "wq": (Wq[:, sl] / 8.0).astype(f16),
            "wk": Wk[:, sl].astype(f16),
            "wv": wv_aug,
            "bq": (bq[sl] / 8.0).astype(np.float32).reshape(D, 1),
            "bk": bk[sl].astype(np.float32).reshape(D, 1),
            "bv": bv_aug,
            "wo": wo_aug,
        })
    return in_maps


LAST_EXEC_NS = None
LAST_TRACE = None


def kernel(**inputs) -> np.ndarray:
    global LAST_EXEC_NS, LAST_TRACE
    trace = bool(int(os.environ.get("KERNEL_TRACE", "0")))
    if trace:
        _install_ntff_hook()
    from concourse.bass_utils import run_bass_kernel_spmd

    nc = _build_program()
    in_maps = _prep_inputs(**{k: np.asarray(v) for k, v in inputs.items()})
    res = run_bass_kernel_spmd(
        nc, in_maps, core_ids=list(range(N_CORES)), trace=trace
    )
    LAST_EXEC_NS = res.exec_time_ns
    LAST_TRACE = res.instructions_and_trace[1] if res.instructions_and_trace else None
    out = res.results[0]["y"].astype(np.float32)
    for c in range(1, N_CORES):
        out += res.results[c]["y"]
    return out


# revision 33
# speedup vs baseline: 1.2482x; 1.0747x over previous
"""Trainium2 Bass kernel for ContMultiHeadedAttention.

Full (unsharded) inputs in, full output out. Sharding: tensor-parallel over
the 8 heads — core c computes head c for both batches and the corresponding
slice of the output projection; the host sums the 8 partial outputs
(row-parallel linear unshard).

Math per head h (performed transposed so no on-device transposes of the
attention matrix are needed):
  qpT = (Wq_h/8).T @ q.T + bq_h/8        [64, S]   (fp16)
  kpT = Wk_h.T @ k.T + bk_h              [64, S]   (fp16)
  vp  = v @ Wv_h + bv_h, augmented with a ones column -> [S, 65]  (fp16)
  sT[k,q] = kpT.T @ qpT  (+ biasT, mask -> -1e4)   (fp32 psum)
  p = exp(sT)  (fp16; no max-subtraction needed: |s| is bounded ~10)
  ctxT[0:64, q] = vp.T @ p (unnormalized), ctxT[64, q] = rowsum
  y = ctxT[0:64].T @ Wo_h + rowsum*bo  then divided by rowsum  [S, 512]
"""

import os
import sys
import types
import numpy as np

B = 2
S = 2048
F = 512          # model dim
H = 8            # heads
D = 64           # head dim
DV = 65          # head dim + ones column
KC = 16          # key chunks of 128 partitions
QC = 4           # query chunks of 512
FC = 4           # feature chunks of 128
N_CORES = 8
NEG = -10000.0   # effective -inf for masked scores (exp underflows to 0)


def _install_ntff_hook():
    """Recreate antenv.axon_hooks if the image lacks it so trace=True works."""
    try:
        import antenv
        if "antenv.axon_hooks" in sys.modules:
            return
        mod = types.ModuleType("antenv.axon_hooks")
        _h = [None]
        mod.set_axon_ntff_profile_hook = lambda h: _h.__setitem__(0, h)
        mod.get_axon_ntff_profile_hook = lambda: _h[0]
        sys.modules["antenv.axon_hooks"] = mod
        antenv.axon_hooks = mod
        from trn_agent_boot.trn_boot import _ntff_profile_via_ctypes
        mod.set_axon_ntff_profile_hook(
            _ntff_profile_via_ctypes("/opt/axon/libaxon_pjrt.so")
        )
    except Exception:
        pass


_PROGRAM = None


def _build_program():
    global _PROGRAM
    if _PROGRAM is not None:
        return _PROGRAM

    import concourse.bacc as bacc
    import concourse.tile as tile
    from concourse import mybir

    f32 = mybir.dt.float32
    f16 = mybir.dt.float16
    u8 = mybir.dt.uint8
    AF = mybir.ActivationFunctionType

    nc = bacc.Bacc("TRN2", target_bir_lowering=False, debug=False,
                   enable_asserts=True, num_devices=N_CORES)

    xq = nc.dram_tensor("xq", [B, F, S], f16, kind="ExternalInput").ap()
    xk = nc.dram_tensor("xk", [B, F, S], f16, kind="ExternalInput").ap()
    xv = nc.dram_tensor("xv", [B, F, S], f16, kind="ExternalInput").ap()
    # biasT[k,q] = bias[q,k] where mask[q,k] else -1e4 (mask folded host-side
    # into the two constant inputs; exp underflows masked scores to 0)
    biasT = nc.dram_tensor("biasT", [S, S], f16, kind="ExternalInput").ap()
    wq_d = nc.dram_tensor("wq", [F, D], f16, kind="ExternalInput").ap()
    wk_d = nc.dram_tensor("wk", [F, D], f16, kind="ExternalInput").ap()
    wv_d = nc.dram_tensor("wv", [F, DV], f16, kind="ExternalInput").ap()
    bq_d = nc.dram_tensor("bq", [D, 1], f32, kind="ExternalInput").ap()
    bk_d = nc.dram_tensor("bk", [D, 1], f32, kind="ExternalInput").ap()
    bv_d = nc.dram_tensor("bv", [1, DV], f32, kind="ExternalInput").ap()
    # wo_aug: [65, F] fp16, row 64 = bo (multiplied by the ones row of cn)
    wo_d = nc.dram_tensor("wo", [DV, F], f16, kind="ExternalInput").ap()
    y_d = nc.dram_tensor("y", [B, S, F], f32, kind="ExternalOutput").ap()

    with tile.TileContext(nc) as tc:
        from contextlib import ExitStack
        with ExitStack() as ctx:
            consts = ctx.enter_context(tc.tile_pool(name="consts", bufs=1))
            persist = ctx.enter_context(tc.tile_pool(name="persist", bufs=1))
            xin = ctx.enter_context(tc.tile_pool(name="xin", bufs=3))
            bmp = ctx.enter_context(tc.tile_pool(name="bmp", bufs=3))
            mkp = ctx.enter_context(tc.tile_pool(name="mkp", bufs=2))
            bms_p = ctx.enter_context(tc.tile_pool(name="bms_p", bufs=4))
            tp = ctx.enter_context(tc.tile_pool(name="tp", bufs=6))
            pp = ctx.enter_context(tc.tile_pool(name="pp", bufs=8))
            cnp = ctx.enter_context(tc.tile_pool(name="cnp", bufs=2))
            rsp = ctx.enter_context(tc.tile_pool(name="rsp", bufs=2))
            rbp = ctx.enter_context(tc.tile_pool(name="rbp", bufs=2))
            psA = ctx.enter_context(tc.tile_pool(name="psA", bufs=2, space="PSUM"))
            psC = ctx.enter_context(tc.tile_pool(name="psC", bufs=1, space="PSUM"))
            psY = ctx.enter_context(tc.tile_pool(name="psY", bufs=2, space="PSUM"))

            # ---- weights / constants in SBUF ----
            wq_sb = consts.tile([128, FC, D], f16, tag="wq")
            nc.sync.dma_start(out=wq_sb[:], in_=wq_d.rearrange("(c p) d -> p c d", p=128))
            wk_sb = consts.tile([128, FC, D], f16, tag="wk")
            nc.sync.dma_start(out=wk_sb[:], in_=wk_d.rearrange("(c p) d -> p c d", p=128))
            wv_sb = consts.tile([128, FC, DV], f16, tag="wv")
            nc.sync.dma_start(out=wv_sb[:], in_=wv_d.rearrange("(c p) d -> p c d", p=128))
            bq_sb = consts.tile([D, 1], f32, tag="bq")
            nc.sync.dma_start(out=bq_sb[:], in_=bq_d[:])
            bk_sb = consts.tile([D, 1], f32, tag="bk")
            nc.sync.dma_start(out=bk_sb[:], in_=bk_d[:])
            wo_sb = consts.tile([DV, F], f16, tag="wo")
            nc.sync.dma_start(out=wo_sb[:], in_=wo_d[:])
            # ones row + bv on the same partition base (K=1 matmuls need
            # lhsT and rhs on the same physical partitions)
            vbias_row = consts.tile([1, 128 + DV], f32, tag="vbias_row")
            nc.gpsimd.memset(vbias_row[:], 1.0)
            nc.sync.dma_start(out=vbias_row[:, 128:128 + DV], in_=bv_d[:])
            ones_row = vbias_row[:, 0:128]
            bv_sb = vbias_row[:, 128:128 + DV]

            qp = {}
            kp = {}
            vp = {}
            for b in range(B):
                qp[b] = persist.tile([D, S], f16, tag=f"qp{b}", name=f"qp{b}")
                kp[b] = persist.tile([D, S], f16, tag=f"kp{b}", name=f"kp{b}")
                vp[b] = persist.tile([128, KC * DV], f16, tag=f"vp{b}", name=f"vp{b}")

            # ---- phase 1: projections ----
            for b in range(B):
                for x_d, w_sb, b_sb, dst in (
                    (xq, wq_sb, bq_sb, qp[b]),
                    (xk, wk_sb, bk_sb, kp[b]),
                ):
                    xt = xin.tile([128, FC, S], f16, tag="xin")
                    nc.sync.dma_start(
                        out=xt[:], in_=x_d[b].rearrange("(c p) s -> p c s", p=128)
                    )
                    for sc in range(2):
                        ps = psA.tile([D, 1024], f32, tag="s")
                        for half in range(2):
                            for fc in range(FC):
                                nc.tensor.matmul(
                                    ps[:, half * 512:(half + 1) * 512],
                                    lhsT=w_sb[:, fc, :],
                                    rhs=xt[:, fc, sc * 1024 + half * 512:
                                           sc * 1024 + (half + 1) * 512],
                                    start=(fc == 0),
                                    stop=(fc == FC - 1),
                                )
                        nc.scalar.activation(
                            dst[:, sc * 1024:(sc + 1) * 1024], ps[:],
                            AF.Identity, bias=b_sb[:],
                        )
                # v projection -> vp [s(128-chunks), 65] with ones column
                xt = xin.tile([128, FC, S], f16, tag="xin")
                nc.sync.dma_start(
                    out=xt[:], in_=xv[b].rearrange("(c p) s -> p c s", p=128)
                )
                for sc in range(KC):
                    ps = psY.tile([128, DV], f32, tag="y", name="psv")
                    for fc in range(FC):
                        nc.tensor.matmul(
                            ps[:],
                            lhsT=xt[:, fc, sc * 128:(sc + 1) * 128],
                            rhs=wv_sb[:, fc, :],
                            start=(fc == 0),
                            stop=False,
                        )
                    nc.tensor.matmul(
                        ps[:], lhsT=ones_row[:], rhs=bv_sb[:],
                        start=False, stop=True,
                    )
                    nc.scalar.activation(
                        vp[b][:, sc * DV:(sc + 1) * DV], ps[:], AF.Copy
                    )

            # ---- phase 2+3: attention + output projection ----
            for qc in range(QC):
                q0 = qc * 512
                bmt = bmp.tile([128, KC * 512], f16, tag="bm")
                nc.sync.dma_start(
                    out=bmt[:],
                    in_=biasT.rearrange("(kc p) q -> p kc q", p=128)[:, :, q0:q0 + 512],
                )
                ctxps = {}
                for b in range(B):
                    ctxps[b] = psC.tile([DV, 512], f32, tag=f"ctx{b}", name=f"ctx{b}")
                for kc in range(KC):
                    # both batches' scores packed in one 2-bank psum tile
                    st2 = psA.tile([128, B * 512], f32, tag="s")
                    for b in range(B):
                        nc.tensor.matmul(
                            st2[:, b * 512:(b + 1) * 512],
                            lhsT=kp[b][:, kc * 128:(kc + 1) * 128],
                            rhs=qp[b][:, q0:q0 + 512],
                            start=True, stop=True,
                        )
                    bms_rep = (
                        bmt[:, kc * 512:(kc + 1) * 512]
                        .rearrange("p (o q) -> p o q", o=1)
                        .broadcast_to((128, B, 512))
                    )
                    tt2 = tp.tile([128, B * 512], f32, tag="t")
                    nc.vector.tensor_add(
                        tt2[:].rearrange("p (o q) -> p o q", o=B),
                        st2[:].rearrange("p (o q) -> p o q", o=B),
                        bms_rep,
                    )
                    pt2 = pp.tile([128, B * 512], f16, tag="p")
                    nc.scalar.activation(pt2[:], tt2[:], AF.Exp)
                    for b in range(B):
                        nc.tensor.matmul(
                            ctxps[b][:],
                            lhsT=vp[b][:, kc * DV:(kc + 1) * DV],
                            rhs=pt2[:, b * 512:(b + 1) * 512],
                            start=(kc == 0),
                            stop=(kc == KC - 1),
                        )
                for b in range(B):
                    # normalize ctx by the rowsum (row 64), then fp16 Wo matmul
                    rr = rsp.tile([1, 512], f32, tag="rr")
                    nc.vector.reciprocal(rr[:], ctxps[b][D:DV, :])
                    rbc = rbp.tile([D, 512], f32, tag="rbc")
                    nc.gpsimd.partition_broadcast(rbc[:], rr[:], channels=D)
                    cn = cnp.tile([DV, 512], f16, tag="cn")
                    nc.vector.tensor_mul(cn[0:D, :], ctxps[b][0:D, :], rbc[:])
                    nc.gpsimd.memset(cn[D:DV, :], 1.0)
                    for s4 in range(4):
                        sl = slice(s4 * 128, (s4 + 1) * 128)
                        yps = psY.tile([128, 512], f32, tag="y", name="yps")
                        nc.tensor.matmul(
                            yps[:], lhsT=cn[:, sl], rhs=wo_sb[:],
                            start=True, stop=True,
                        )
                        yt = tp.tile([128, 512], f32, tag="yt")
                        nc.scalar.activation(yt[:], yps[:], AF.Copy)
                        nc.sync.dma_start(
                            out=y_d[b, q0 + s4 * 128:q0 + (s4 + 1) * 128, :],
                            in_=yt[:],
                        )

    nc.compile()
    _PROGRAM = nc
    return nc


def _prep_inputs(k, v, q, mask, spatial_bias, Wq, bq, Wk, bk, Wv, bv, Wo, bo):
    """Build the 8 per-core input maps (host-side sharding / layout only)."""
    f16 = np.float16
    qT = np.ascontiguousarray(np.transpose(q, (0, 2, 1)).astype(f16))
    kT = np.ascontiguousarray(np.transpose(k, (0, 2, 1)).astype(f16))
    vT = np.ascontiguousarray(np.transpose(v, (0, 2, 1)).astype(f16))
    maskT = mask.T
    negf = np.float16(NEG)

    in_maps = []
    for h in range(N_CORES):
        sl = slice(h * D, (h + 1) * D)
        wv_aug = np.concatenate(
            [Wv[:, sl], np.zeros((F, 1), np.float32)], axis=1
        ).astype(f16)
        bv_aug = np.concatenate([bv[sl], [1.0]]).astype(np.float32).reshape(1, DV)
        bo_h = bo if h == 0 else np.zeros_like(bo)
        wo_aug = np.concatenate(
            [Wo[sl, :], bo_h.reshape(1, F)], axis=0
        ).astype(f16)
        in_maps.append({
            "xq": qT, "xk": kT, "xv": vT,
            "biasT": np.where(maskT, spatial_bias[0, h].T.astype(f16), negf),
            "wq": (Wq[:, sl] / 8.0).astype(f16),
            "wk": Wk[:, sl].astype(f16),
            "wv": wv_aug,
            "bq": (bq[sl] / 8.0).astype(np.float32).reshape(D, 1),
            "bk": bk[sl].astype(np.float32).reshape(D, 1),
            "bv": bv_aug,
            "wo": wo_aug,
        })
    return in_maps


LAST_EXEC_NS = None
LAST_TRACE = None


def kernel(**inputs) -> np.ndarray:
    global LAST_EXEC_NS, LAST_TRACE
    trace = bool(int(os.environ.get("KERNEL_TRACE", "0")))
    if trace:
        _install_ntff_hook()
    from concourse.bass_utils import run_bass_kernel_spmd

    nc = _build_program()
    in_maps = _prep_inputs(**{k: np.asarray(v) for k, v in inputs.items()})
    res = run_bass_kernel_spmd(
        nc, in_maps, core_ids=list(range(N_CORES)), trace=trace
    )
    LAST_EXEC_NS = res.exec_time_ns
    LAST_TRACE = res.instructions_and_trace[1] if res.instructions_and_trace else None
    out = res.results[0]["y"].astype(np.float32)
    for c in range(1, N_CORES):
        out += res.results[c]["y"]
    return out


# revision 34
# speedup vs baseline: 1.3557x; 1.0861x over previous
"""Trainium2 Bass kernel for ContMultiHeadedAttention.

Full (unsharded) inputs in, full output out. Sharding: tensor-parallel over
the 8 heads — core c computes head c for both batches and the corresponding
slice of the output projection; the host sums the 8 partial outputs
(row-parallel linear unshard).

Math per head h (performed transposed so no on-device transposes of the
attention matrix are needed):
  qpT = (Wq_h/8).T @ q.T + bq_h/8        [64, S]   (fp16)
  kpT = Wk_h.T @ k.T + bk_h              [64, S]   (fp16)
  vp  = v @ Wv_h + bv_h, augmented with a ones column -> [S, 65]  (fp16)
  sT[k,q] = kpT.T @ qpT  (+ biasT, mask -> -1e4)   (fp32 psum)
  p = exp(sT)  (fp16; no max-subtraction needed: |s| is bounded ~10)
  ctxT[0:64, q] = vp.T @ p (unnormalized), ctxT[64, q] = rowsum
  y = ctxT[0:64].T @ Wo_h + rowsum*bo  then divided by rowsum  [S, 512]
"""

import os
import sys
import types
import numpy as np

B = 2
S = 2048
F = 512          # model dim
H = 8            # heads
D = 64           # head dim
DV = 65          # head dim + ones column
KC = 16          # key chunks of 128 partitions
QC = 4           # query chunks of 512
FC = 4           # feature chunks of 128
N_CORES = 8
NEG = -10000.0   # effective -inf for masked scores (exp underflows to 0)


def _install_ntff_hook():
    """Recreate antenv.axon_hooks if the image lacks it so trace=True works."""
    try:
        import antenv
        if "antenv.axon_hooks" in sys.modules:
            return
        mod = types.ModuleType("antenv.axon_hooks")
        _h = [None]
        mod.set_axon_ntff_profile_hook = lambda h: _h.__setitem__(0, h)
        mod.get_axon_ntff_profile_hook = lambda: _h[0]
        sys.modules["antenv.axon_hooks"] = mod
        antenv.axon_hooks = mod
        from trn_agent_boot.trn_boot import _ntff_profile_via_ctypes
        mod.set_axon_ntff_profile_hook(
            _ntff_profile_via_ctypes("/opt/axon/libaxon_pjrt.so")
        )
    except Exception:
        pass


_PROGRAM = None


def _build_program():
    global _PROGRAM
    if _PROGRAM is not None:
        return _PROGRAM

    import concourse.bacc as bacc
    import concourse.tile as tile
    from concourse import mybir

    f32 = mybir.dt.float32
    f16 = mybir.dt.float16
    u8 = mybir.dt.uint8
    AF = mybir.ActivationFunctionType

    nc = bacc.Bacc("TRN2", target_bir_lowering=False, debug=False,
                   enable_asserts=True, num_devices=N_CORES)

    xq = nc.dram_tensor("xq", [B, F, S], f16, kind="ExternalInput").ap()
    xk = nc.dram_tensor("xk", [B, F, S], f16, kind="ExternalInput").ap()
    xv = nc.dram_tensor("xv", [B, F, S], f16, kind="ExternalInput").ap()
    # biasT[k,q] = bias[q,k] where mask[q,k] else -1e4 (mask folded host-side
    # into the two constant inputs; exp underflows masked scores to 0)
    biasT = nc.dram_tensor("biasT", [S, S], f16, kind="ExternalInput").ap()
    wq_d = nc.dram_tensor("wq", [F, D], f16, kind="ExternalInput").ap()
    wk_d = nc.dram_tensor("wk", [F, D], f16, kind="ExternalInput").ap()
    wv_d = nc.dram_tensor("wv", [F, DV], f16, kind="ExternalInput").ap()
    bq_d = nc.dram_tensor("bq", [D, 1], f32, kind="ExternalInput").ap()
    bk_d = nc.dram_tensor("bk", [D, 1], f32, kind="ExternalInput").ap()
    bv_d = nc.dram_tensor("bv", [1, DV], f32, kind="ExternalInput").ap()
    # wo_aug: [65, F] fp16, row 64 = bo (multiplied by the ones row of cn)
    wo_d = nc.dram_tensor("wo", [DV, F], f16, kind="ExternalInput").ap()
    y_d = nc.dram_tensor("y", [B, S, F], f32, kind="ExternalOutput").ap()

    with tile.TileContext(nc) as tc:
        from contextlib import ExitStack
        with ExitStack() as ctx:
            consts = ctx.enter_context(tc.tile_pool(name="consts", bufs=1))
            persist = ctx.enter_context(tc.tile_pool(name="persist", bufs=1))
            xin = ctx.enter_context(tc.tile_pool(name="xin", bufs=3))
            bmp = ctx.enter_context(tc.tile_pool(name="bmp", bufs=3))
            mkp = ctx.enter_context(tc.tile_pool(name="mkp", bufs=2))
            bms_p = ctx.enter_context(tc.tile_pool(name="bms_p", bufs=4))
            tp = ctx.enter_context(tc.tile_pool(name="tp", bufs=6))
            pp = ctx.enter_context(tc.tile_pool(name="pp", bufs=8))
            cnp = ctx.enter_context(tc.tile_pool(name="cnp", bufs=2))
            rsp = ctx.enter_context(tc.tile_pool(name="rsp", bufs=2))
            rbp = ctx.enter_context(tc.tile_pool(name="rbp", bufs=2))
            psA = ctx.enter_context(tc.tile_pool(name="psA", bufs=2, space="PSUM"))
            psC = ctx.enter_context(tc.tile_pool(name="psC", bufs=1, space="PSUM"))
            psY = ctx.enter_context(tc.tile_pool(name="psY", bufs=2, space="PSUM"))

            # ---- weights / constants in SBUF ----
            wq_sb = consts.tile([128, FC, D], f16, tag="wq")
            nc.sync.dma_start(out=wq_sb[:], in_=wq_d.rearrange("(c p) d -> p c d", p=128))
            wk_sb = consts.tile([128, FC, D], f16, tag="wk")
            nc.sync.dma_start(out=wk_sb[:], in_=wk_d.rearrange("(c p) d -> p c d", p=128))
            wv_sb = consts.tile([128, FC, DV], f16, tag="wv")
            nc.sync.dma_start(out=wv_sb[:], in_=wv_d.rearrange("(c p) d -> p c d", p=128))
            bq_sb = consts.tile([D, 1], f32, tag="bq")
            nc.sync.dma_start(out=bq_sb[:], in_=bq_d[:])
            bk_sb = consts.tile([D, 1], f32, tag="bk")
            nc.sync.dma_start(out=bk_sb[:], in_=bk_d[:])
            wo_sb = consts.tile([DV, F], f16, tag="wo")
            nc.sync.dma_start(out=wo_sb[:], in_=wo_d[:])
            # ones row + bv on the same partition base (K=1 matmuls need
            # lhsT and rhs on the same physical partitions)
            vbias_row = consts.tile([1, 128 + DV], f32, tag="vbias_row")
            nc.gpsimd.memset(vbias_row[:], 1.0)
            nc.sync.dma_start(out=vbias_row[:, 128:128 + DV], in_=bv_d[:])
            ones_row = vbias_row[:, 0:128]
            bv_sb = vbias_row[:, 128:128 + DV]

            qp = {}
            kp = {}
            vp = {}
            for b in range(B):
                qp[b] = persist.tile([D, S], f16, tag=f"qp{b}", name=f"qp{b}")
                kp[b] = persist.tile([D, S], f16, tag=f"kp{b}", name=f"kp{b}")
                vp[b] = persist.tile([128, KC * DV], f16, tag=f"vp{b}", name=f"vp{b}")

            # ---- phase 1: projections ----
            for b in range(B):
                for x_d, w_sb, b_sb, dst in (
                    (xq, wq_sb, bq_sb, qp[b]),
                    (xk, wk_sb, bk_sb, kp[b]),
                ):
                    xt = xin.tile([128, FC, S], f16, tag="xin")
                    nc.sync.dma_start(
                        out=xt[:], in_=x_d[b].rearrange("(c p) s -> p c s", p=128)
                    )
                    for sc in range(2):
                        ps = psA.tile([D, 1024], f32, tag="s")
                        for half in range(2):
                            for fc in range(FC):
                                nc.tensor.matmul(
                                    ps[:, half * 512:(half + 1) * 512],
                                    lhsT=w_sb[:, fc, :],
                                    rhs=xt[:, fc, sc * 1024 + half * 512:
                                           sc * 1024 + (half + 1) * 512],
                                    start=(fc == 0),
                                    stop=(fc == FC - 1),
                                )
                        nc.scalar.activation(
                            dst[:, sc * 1024:(sc + 1) * 1024], ps[:],
                            AF.Identity, bias=b_sb[:],
                        )
                # v projection -> vp [s(128-chunks), 65] with ones column
                xt = xin.tile([128, FC, S], f16, tag="xin")
                nc.sync.dma_start(
                    out=xt[:], in_=xv[b].rearrange("(c p) s -> p c s", p=128)
                )
                for sc in range(KC):
                    ps = psY.tile([128, DV], f32, tag="y", name="psv")
                    for fc in range(FC):
                        nc.tensor.matmul(
                            ps[:],
                            lhsT=xt[:, fc, sc * 128:(sc + 1) * 128],
                            rhs=wv_sb[:, fc, :],
                            start=(fc == 0),
                            stop=False,
                        )
                    nc.tensor.matmul(
                        ps[:], lhsT=ones_row[:], rhs=bv_sb[:],
                        start=False, stop=True,
                    )
                    nc.scalar.activation(
                        vp[b][:, sc * DV:(sc + 1) * DV], ps[:], AF.Copy
                    )

            # ---- phase 2+3: attention + output projection ----
            # Software-pipelined: scores run 2 kc ahead of ctx in the PE
            # stream; rowsum recip via ACT exp(-ln(x)); y-phase of qc is
            # emitted inside qc+1's prologue so the PE never idles on it.

            def emit_scores(q0, kc):
                st2 = psA.tile([128, B * 512], f32, tag="s", name="st2")
                for b in range(B):
                    nc.tensor.matmul(
                        st2[:, b * 512:(b + 1) * 512],
                        lhsT=kp[b][:, kc * 128:(kc + 1) * 128],
                        rhs=qp[b][:, q0:q0 + 512],
                        start=True, stop=True,
                    )
                return st2

            def emit_attn_step(q0, kc, bmt, ctxps, sts):
                st2 = sts.pop(kc)
                bms_rep = (
                    bmt[:, kc * 512:(kc + 1) * 512]
                    .rearrange("p (o q) -> p o q", o=1)
                    .broadcast_to((128, B, 512))
                )
                tt2 = tp.tile([128, B * 512], f32, tag="t", name="tt2")
                nc.vector.tensor_add(
                    tt2[:].rearrange("p (o q) -> p o q", o=B),
                    st2[:].rearrange("p (o q) -> p o q", o=B),
                    bms_rep,
                )
                pt2 = pp.tile([128, B * 512], f16, tag="p", name="pt2")
                nc.scalar.activation(pt2[:], tt2[:], AF.Exp)
                for b in range(B):
                    nc.tensor.matmul(
                        ctxps[b][:],
                        lhsT=vp[b][:, kc * DV:(kc + 1) * DV],
                        rhs=pt2[:, b * 512:(b + 1) * 512],
                        start=(kc == 0),
                        stop=(kc == KC - 1),
                    )
                if kc + 2 < KC:
                    sts[kc + 2] = emit_scores(q0, kc + 2)

            def emit_norm(ctxps):
                cns = []
                for b in range(B):
                    lnr = rsp.tile([1, 512], f32, tag="lnr", name="lnr")
                    nc.scalar.activation(lnr[:], ctxps[b][D:DV, :], AF.Ln)
                    rr = rsp.tile([1, 512], f32, tag="rr", name="rr")
                    nc.scalar.activation(rr[:], lnr[:], AF.Exp, scale=-1.0)
                    rbc = rbp.tile([D, 512], f32, tag="rbc", name="rbc")
                    nc.gpsimd.partition_broadcast(rbc[:], rr[:], channels=D)
                    cn = cnp.tile([DV, 512], f16, tag="cn", name="cn")
                    nc.vector.tensor_mul(cn[0:D, :], ctxps[b][0:D, :], rbc[:])
                    nc.gpsimd.memset(cn[D:DV, :], 1.0)
                    cns.append(cn)
                return cns

            def emit_y(q0, cns):
                for b in range(B):
                    for s4 in range(4):
                        sl = slice(s4 * 128, (s4 + 1) * 128)
                        yps = psY.tile([128, 512], f32, tag="y", name="yps")
                        nc.tensor.matmul(
                            yps[:], lhsT=cns[b][:, sl], rhs=wo_sb[:],
                            start=True, stop=True,
                        )
                        yt = tp.tile([128, 512], f32, tag="yt", name="yt")
                        nc.scalar.activation(yt[:], yps[:], AF.Copy)
                        nc.sync.dma_start(
                            out=y_d[b, q0 + s4 * 128:q0 + (s4 + 1) * 128, :],
                            in_=yt[:],
                        )

            carry = None
            for qc in range(QC):
                q0 = qc * 512
                bmt = bmp.tile([128, KC * 512], f16, tag="bm", name="bmt")
                nc.sync.dma_start(
                    out=bmt[:],
                    in_=biasT.rearrange("(kc p) q -> p kc q", p=128)[:, :, q0:q0 + 512],
                )
                ctxps = {}
                for b in range(B):
                    ctxps[b] = psC.tile([DV, 512], f32, tag=f"ctx{b}", name=f"ctx{b}")
                sts = {0: emit_scores(q0, 0), 1: emit_scores(q0, 1)}
                emit_attn_step(q0, 0, bmt, ctxps, sts)
                emit_attn_step(q0, 1, bmt, ctxps, sts)
                if carry is not None:
                    emit_y(*carry)
                    carry = None
                for kc in range(2, KC):
                    emit_attn_step(q0, kc, bmt, ctxps, sts)
                carry = (q0, emit_norm(ctxps))
            emit_y(*carry)

    nc.compile()
    _PROGRAM = nc
    return nc


def _prep_inputs(k, v, q, mask, spatial_bias, Wq, bq, Wk, bk, Wv, bv, Wo, bo):
    """Build the 8 per-core input maps (host-side sharding / layout only)."""
    f16 = np.float16
    qT = np.ascontiguousarray(np.transpose(q, (0, 2, 1)).astype(f16))
    kT = np.ascontiguousarray(np.transpose(k, (0, 2, 1)).astype(f16))
    vT = np.ascontiguousarray(np.transpose(v, (0, 2, 1)).astype(f16))
    maskT = mask.T
    negf = np.float16(NEG)

    in_maps = []
    for h in range(N_CORES):
        sl = slice(h * D, (h + 1) * D)
        wv_aug = np.concatenate(
            [Wv[:, sl], np.zeros((F, 1), np.float32)], axis=1
        ).astype(f16)
        bv_aug = np.concatenate([bv[sl], [1.0]]).astype(np.float32).reshape(1, DV)
        bo_h = bo if h == 0 else np.zeros_like(bo)
        wo_aug = np.concatenate(
            [Wo[sl, :], bo_h.reshape(1, F)], axis=0
        ).astype(f16)
        in_maps.append({
            "xq": qT, "xk": kT, "xv": vT,
            "biasT": np.where(maskT, spatial_bias[0, h].T.astype(f16), negf),
            "wq": (Wq[:, sl] / 8.0).astype(f16),
            "wk": Wk[:, sl].astype(f16),
            "wv": wv_aug,
            "bq": (bq[sl] / 8.0).astype(np.float32).reshape(D, 1),
            "bk": bk[sl].astype(np.float32).reshape(D, 1),
            "bv": bv_aug,
            "wo": wo_aug,
        })
    return in_maps


LAST_EXEC_NS = None
LAST_TRACE = None


def kernel(**inputs) -> np.ndarray:
    global LAST_EXEC_NS, LAST_TRACE
    trace = bool(int(os.environ.get("KERNEL_TRACE", "0")))
    if trace:
        _install_ntff_hook()
    from concourse.bass_utils import run_bass_kernel_spmd

    nc = _build_program()
    in_maps = _prep_inputs(**{k: np.asarray(v) for k, v in inputs.items()})
    res = run_bass_kernel_spmd(
        nc, in_maps, core_ids=list(range(N_CORES)), trace=trace
    )
    LAST_EXEC_NS = res.exec_time_ns
    LAST_TRACE = res.instructions_and_trace[1] if res.instructions_and_trace else None
    out = res.results[0]["y"].astype(np.float32)
    for c in range(1, N_CORES):
        out += res.results[c]["y"]
    return out
